# revision 1
# baseline (speedup 1.0000x reference)
"""Bass/Trainium2 kernel for 2-layer GAT (nn_GAT_50577534878113).

Strategy (8 NeuronCores, SPMD):
  - Nodes padded to NP = NBLK*128; dst-sorted edges sharded by dst-block range:
    core k owns BPC = NBLK/8 blocks of 128 destination nodes.
  - Dense phases (x@W1 etc.) replicated per core in bf16 (cheap on PE); the
    per-node payload table [h | a_src] is written to a per-core DRAM table.
  - Edge phase per 128-edge chunk (dst-block local): one K=1 indirect-DMA
    gather of payload rows by src id; one-hot matrices built on-chip
    (iota vs dst_rel is_equal) route a_dst expansion and the scatter-add as
    TensorE matmuls accumulating in PSUM per dst block. Softmax is computed
    without max-subtraction (logits are O(10), fp32 exp is exact enough) so
    denominators are aggregated alongside messages in the same matmuls.
  - Layer-2 local dense from the (transposed) layer-1 block outputs, then one
    AllGather distributes the global layer-2 payload table; the layer-2 edge
    phase mirrors layer 1. Output is node-sharded, host concatenates.

To keep per-core programs identical (SPMD), each core's node table is block-
rotated so its own 49 dst blocks come first; L1 gather indices are rotated to
match. The AllGather (in core order) restores the global node order for L2.
"""

import numpy as np
import ml_dtypes

bf16 = ml_dtypes.bfloat16

# Problem shapes (hardcoded per contract)
N_NODES = 50000
N_EDGES = 800000
IN_CH = 128
HEADS = 4
HIDDEN = 32
OUT_CH = 40
NEG = 0.2
NCORES = 8
BLK = 128

F1 = IN_CH + HEADS          # 132: [h1 (128) | a_src1 (4)]
F1T = F1 + HEADS            # 136: + a_dst1 (4)
F2 = OUT_CH + 2             # 42:  [h2 (40) | a_src2 | a_dst2]


def _build(NP, NBLK, BPC, CPB):
    import concourse.bass as bass
    import concourse.bacc as bacc
    import concourse.mybir as mybir
    import concourse.tile as tile

    dt = mybir.dt
    AL = mybir.AluOpType
    AF = mybir.ActivationFunctionType

    nc = bacc.Bacc("TRN2", target_bir_lowering=False, debug=False,
                   num_devices=NCORES)

    XT = nc.dram_tensor("xt", [128, NP], dt.bfloat16, kind="ExternalInput").ap()
    W1A = nc.dram_tensor("w1a", [128, F1T], dt.bfloat16, kind="ExternalInput").ap()
    W2A = nc.dram_tensor("w2a", [128, F2], dt.bfloat16, kind="ExternalInput").ap()
    IOTA = nc.dram_tensor("iota", [128, 128], dt.bfloat16, kind="ExternalInput").ap()
    IDB = nc.dram_tensor("idb", [128, 128], dt.bfloat16, kind="ExternalInput").ap()
    IDF = nc.dram_tensor("idf", [128, 128], dt.float32, kind="ExternalInput").ap()
    HSEL = nc.dram_tensor("hsel", [HEADS, 128], dt.bfloat16, kind="ExternalInput").ap()
    ONES1 = nc.dram_tensor("ones1", [1, OUT_CH], dt.bfloat16, kind="ExternalInput").ap()
    SRC1 = nc.dram_tensor("src1", [BPC, 128, CPB], dt.int32, kind="ExternalInput").ap()
    SRC2 = nc.dram_tensor("src2", [BPC, 128, CPB], dt.int32, kind="ExternalInput").ap()
    DREL = nc.dram_tensor("drel", [BPC, 128, CPB], dt.bfloat16, kind="ExternalInput").ap()
    OUT = nc.dram_tensor("out", [BPC * 128, OUT_CH], dt.float32, kind="ExternalOutput").ap()

    PL1 = nc.dram_tensor("pl1", [NP, F1], dt.bfloat16).ap()
    L2L = nc.dram_tensor("l2l", [BPC * 128, F2], dt.bfloat16).ap()
    PL2 = nc.dram_tensor("pl2", [NP, F2], dt.bfloat16, addr_space="Shared").ap()

    with tile.TileContext(nc) as tc:
        with tc.tile_pool(name="const", bufs=1) as cp, \
             tc.tile_pool(name="sb", bufs=3) as sp, \
             tc.tile_pool(name="blk", bufs=3) as bp, \
             tc.tile_pool(name="ps", bufs=2, space="PSUM") as pp:

            iota = cp.tile([128, 128], dt.bfloat16)
            nc.sync.dma_start(out=iota[:], in_=IOTA[:])
            idb = cp.tile([128, 128], dt.bfloat16)
            nc.sync.dma_start(out=idb[:], in_=IDB[:])
            idf = cp.tile([128, 128], dt.float32)
            nc.sync.dma_start(out=idf[:], in_=IDF[:])
            hsel = cp.tile([HEADS, 128], dt.bfloat16)
            nc.sync.dma_start(out=hsel[:], in_=HSEL[:])
            ones1 = cp.tile([1, OUT_CH], dt.bfloat16)
            nc.sync.dma_start(out=ones1[:], in_=ONES1[:])
            w1a = cp.tile([128, F1T], dt.bfloat16)
            nc.sync.dma_start(out=w1a[:], in_=W1A[:])
            w2a = cp.tile([128, F2], dt.bfloat16)
            nc.sync.dma_start(out=w2a[:], in_=W2A[:])

            adst1 = cp.tile([128, BPC * HEADS], dt.bfloat16)
            adst2 = cp.tile([128, BPC], dt.bfloat16)
            elu1t = cp.tile([128, BPC * 128], dt.bfloat16)

            # ---- stage 1: dense h1/a_src1/a_dst1 for ALL nodes; write PL1 ----
            XCH = 49 if NBLK % 49 == 0 else 1   # blocks per big x load
            for g0 in range(0, NBLK, XCH):
                xt_big = sp.tile([128, XCH * 128], dt.bfloat16, tag="xt")
                nc.sync.dma_start(out=xt_big[:],
                                  in_=XT[:, g0 * 128:(g0 + XCH) * 128])
                for j in range(XCH):
                    nb = g0 + j
                    dps = pp.tile([128, F1T], dt.float32,
                                  tag=["agg", "trq", "den", "adg"][nb % 4])
                    nc.tensor.matmul(dps[:], lhsT=xt_big[:, j * 128:(j + 1) * 128],
                                     rhs=w1a[:], start=True, stop=True)
                    sb1 = sp.tile([128, F1], dt.bfloat16, tag=f"pl1sb{nb % 2}")
                    nc.vector.tensor_copy(sb1[:], dps[:, 0:F1])
                    eng = nc.scalar if nb % 2 else nc.sync
                    eng.dma_start(out=PL1[nb * 128:(nb + 1) * 128, :], in_=sb1[:])
                    if nb < BPC:
                        nc.vector.tensor_copy(adst1[:, nb * HEADS:(nb + 1) * HEADS],
                                              dps[:, F1:F1T])

            # ---- stage 2: layer-1 edge pass over own dst blocks ----
            for b in range(BPC):
                idx_t = sp.tile([128, CPB], dt.int32, tag="idx")
                nc.sync.dma_start(out=idx_t[:], in_=SRC1[b])
                dr_t = sp.tile([128, CPB], dt.bfloat16, tag="dr")
                nc.sync.dma_start(out=dr_t[:], in_=DREL[b])

                g = bp.tile([128, CPB * F1], dt.bfloat16, tag="g")
                for c in range(CPB):
                    nc.gpsimd.indirect_dma_start(
                        out=g[:, c * F1:(c + 1) * F1], out_offset=None,
                        in_=PL1[:],
                        in_offset=bass.IndirectOffsetOnAxis(ap=idx_t[:, c:c + 1], axis=0))

                # one-hot P[e, c, j] = (dst_rel[e,c] == j), all chunks at once
                P = bp.tile([128, CPB * 128], dt.bfloat16, tag="P")
                nc.vector.tensor_tensor(
                    out=P[:].rearrange("p (c j) -> p c j", j=128),
                    in0=dr_t[:, :, None].broadcast_to([128, CPB, 128]),
                    in1=iota[:, None, :].broadcast_to([128, CPB, 128]),
                    op=AL.is_equal)

                ADG = pp.tile([128, CPB * HEADS], dt.float32, tag="adg")
                AGG = pp.tile([128, 128], dt.float32, tag="agg")
                DEN = pp.tile([HEADS, 128], dt.float32, tag="den")

                for c in range(CPB):
                    trq = pp.tile([128, 128], dt.bfloat16, tag="trq")
                    nc.tensor.transpose(out=trq[:], in_=P[:, c * 128:(c + 1) * 128],
                                        identity=idb[:])
                    qd = sp.tile([128, 128], dt.bfloat16, tag="qd")
                    nc.scalar.copy(out=qd[:], in_=trq[:])
                    nc.tensor.matmul(ADG[:, c * HEADS:(c + 1) * HEADS], lhsT=qd[:],
                                     rhs=adst1[:, b * HEADS:(b + 1) * HEADS],
                                     start=True, stop=True)

                # logits, leaky-relu, exp (batched over the block's chunks)
                elog = sp.tile([128, CPB * HEADS], dt.float32, tag="elog")
                nc.vector.tensor_tensor(
                    out=elog[:].rearrange("p (c f) -> p c f", f=HEADS),
                    in0=ADG[:].rearrange("p (c f) -> p c f", f=HEADS),
                    in1=g[:].rearrange("p (c f) -> p c f", f=F1)[:, :, IN_CH:IN_CH + HEADS],
                    op=AL.add)
                lr = sp.tile([128, CPB * HEADS], dt.float32, tag="lr")
                nc.vector.tensor_scalar(out=lr[:], in0=elog[:], scalar1=NEG,
                                        scalar2=None, op0=AL.mult)
                nc.vector.tensor_tensor(out=lr[:], in0=lr[:], in1=elog[:], op=AL.max)
                s_all = sp.tile([128, CPB * HEADS], dt.bfloat16, tag="sall")
                nc.scalar.activation(out=s_all[:], in_=lr[:], func=AF.Exp)

                for c in range(CPB):
                    gs = sp.tile([128, 128], dt.bfloat16, tag="gs")
                    nc.vector.tensor_tensor(
                        out=gs[:].rearrange("p (h w) -> p h w", w=HIDDEN),
                        in0=g[:, c * F1:c * F1 + IN_CH].rearrange("p (h w) -> p h w", w=HIDDEN),
                        in1=s_all[:, c * HEADS:(c + 1) * HEADS][:, :, None]
                            .broadcast_to([128, HEADS, HIDDEN]),
                        op=AL.mult)
                    nc.tensor.matmul(AGG[:], lhsT=gs[:], rhs=P[:, c * 128:(c + 1) * 128],
                                     start=(c == 0), stop=(c == CPB - 1))
                    nc.tensor.matmul(DEN[:], lhsT=s_all[:, c * HEADS:(c + 1) * HEADS],
                                     rhs=P[:, c * 128:(c + 1) * 128],
                                     start=(c == 0), stop=(c == CPB - 1))

                # normalize + ELU, store transposed block to elu1t
                den_sb = sp.tile([HEADS, 128], dt.float32, tag="densb")
                nc.vector.tensor_scalar(out=den_sb[:], in0=DEN[:], scalar1=1e-16,
                                        scalar2=None, op0=AL.add)
                rec = sp.tile([HEADS, 128], dt.float32, tag="rec")
                nc.vector.reciprocal_approx_fast(out=rec[:], in_=den_sb[:])
                rec_bf = sp.tile([HEADS, 128], dt.bfloat16, tag="recbf")
                nc.vector.tensor_copy(rec_bf[:], rec[:])
                rep = pp.tile([128, 128], dt.float32, tag="trq")
                nc.tensor.matmul(rep[:], lhsT=hsel[:], rhs=rec_bf[:], start=True, stop=True)
                rep_sb = sp.tile([128, 128], dt.float32, tag="repsb")
                nc.scalar.copy(out=rep_sb[:], in_=rep[:])
                t1 = sp.tile([128, 128], dt.float32, tag="t1")
                nc.vector.tensor_tensor(out=t1[:], in0=AGG[:], in1=rep_sb[:], op=AL.mult)
                m1 = sp.tile([128, 128], dt.float32, tag="m1")
                nc.vector.tensor_scalar(out=m1[:], in0=t1[:], scalar1=0.0,
                                        scalar2=None, op0=AL.min)
                u1 = sp.tile([128, 128], dt.float32, tag="u1")
                nc.scalar.activation(out=u1[:], in_=m1[:], func=AF.Exp)
                pm1 = sp.tile([128, 128], dt.float32, tag="pm1")
                nc.vector.tensor_scalar(out=pm1[:], in0=t1[:], scalar1=0.0, scalar2=-1.0,
                                        op0=AL.max, op1=AL.add)
                nc.vector.tensor_tensor(out=elu1t[:, b * 128:(b + 1) * 128],
                                        in0=pm1[:], in1=u1[:], op=AL.add)

                # layer-2 local dense for this block (fused stage 3)
                d2 = pp.tile([128, F2], dt.float32, tag="adg")
                nc.tensor.matmul(d2[:], lhsT=elu1t[:, b * 128:(b + 1) * 128],
                                 rhs=w2a[:], start=True, stop=True)
                sb2 = sp.tile([128, F2], dt.bfloat16, tag="sb2")
                nc.scalar.copy(out=sb2[:], in_=d2[:])
                nc.scalar.dma_start(out=L2L[b * 128:(b + 1) * 128, :], in_=sb2[:])
                nc.vector.tensor_copy(adst2[:, b:b + 1], d2[:, F2 - 1:F2])

            nc.gpsimd.collective_compute(
                "AllGather", mybir.AluOpType.bypass,
                replica_groups=[list(range(NCORES))],
                ins=[L2L[:, :]], outs=[PL2[:, :]])

            # ---- stage 4: layer-2 edge pass ----
            for b in range(BPC):
                idx2 = sp.tile([128, CPB], dt.int32, tag="idx")
                nc.sync.dma_start(out=idx2[:], in_=SRC2[b])
                dr_t = sp.tile([128, CPB], dt.bfloat16, tag="dr")
                nc.sync.dma_start(out=dr_t[:], in_=DREL[b])

                g2 = bp.tile([128, CPB * F2], dt.bfloat16, tag="g2")
                for c in range(CPB):
                    nc.gpsimd.indirect_dma_start(
                        out=g2[:, c * F2:(c + 1) * F2], out_offset=None,
                        in_=PL2[:],
                        in_offset=bass.IndirectOffsetOnAxis(ap=idx2[:, c:c + 1], axis=0))

                P = bp.tile([128, CPB * 128], dt.bfloat16, tag="P")
                nc.vector.tensor_tensor(
                    out=P[:].rearrange("p (c j) -> p c j", j=128),
                    in0=dr_t[:, :, None].broadcast_to([128, CPB, 128]),
                    in1=iota[:, None, :].broadcast_to([128, CPB, 128]),
                    op=AL.is_equal)

                ADG2 = pp.tile([128, CPB], dt.float32, tag="adg")
                AGG2 = pp.tile([OUT_CH, 128], dt.float32, tag="agg")

                for c in range(CPB):
                    trq = pp.tile([128, 128], dt.bfloat16, tag="trq")
                    nc.tensor.transpose(out=trq[:], in_=P[:, c * 128:(c + 1) * 128],
                                        identity=idb[:])
                    qd = sp.tile([128, 128], dt.bfloat16, tag="qd")
                    nc.scalar.copy(out=qd[:], in_=trq[:])
                    nc.tensor.matmul(ADG2[:, c:c + 1], lhsT=qd[:],
                                     rhs=adst2[:, b:b + 1], start=True, stop=True)

                elog2 = sp.tile([128, CPB], dt.float32, tag="elog")
                nc.vector.tensor_tensor(
                    out=elog2[:, :, None],
                    in0=ADG2[:, :, None],
                    in1=g2[:].rearrange("p (c f) -> p c f", f=F2)[:, :, OUT_CH:OUT_CH + 1],
                    op=AL.add)
                lr2 = sp.tile([128, CPB], dt.float32, tag="lr")
                nc.vector.tensor_scalar(out=lr2[:], in0=elog2[:], scalar1=NEG,
                                        scalar2=None, op0=AL.mult)
                nc.vector.tensor_tensor(out=lr2[:], in0=lr2[:], in1=elog2[:], op=AL.max)
                s2 = sp.tile([128, CPB], dt.float32, tag="sall2")
                nc.scalar.activation(out=s2[:], in_=lr2[:], func=AF.Exp)
                s2b = sp.tile([128, CPB], dt.bfloat16, tag="sall")
                nc.scalar.copy(out=s2b[:], in_=s2[:])

                DEN2 = pp.tile([1, 128], dt.float32, tag="den")
                for c in range(CPB):
                    gs2 = sp.tile([128, OUT_CH], dt.bfloat16, tag="gs")
                    nc.vector.tensor_scalar(out=gs2[:],
                                            in0=g2[:, c * F2:c * F2 + OUT_CH],
                                            scalar1=s2[:, c:c + 1], scalar2=None,
                                            op0=AL.mult)
                    nc.tensor.matmul(AGG2[:], lhsT=gs2[:], rhs=P[:, c * 128:(c + 1) * 128],
                                     start=(c == 0), stop=(c == CPB - 1))
                    nc.tensor.matmul(DEN2[:], lhsT=s2b[:, c:c + 1],
                                     rhs=P[:, c * 128:(c + 1) * 128],
                                     start=(c == 0), stop=(c == CPB - 1))

                ag2sb = sp.tile([OUT_CH, 128], dt.float32, tag="ag2sb")
                nc.scalar.copy(out=ag2sb[:], in_=AGG2[:])
                den2 = sp.tile([1, 128], dt.float32, tag="densb")
                nc.vector.tensor_scalar(out=den2[:], in0=DEN2[:],
                                        scalar1=1e-16, scalar2=None, op0=AL.add)
                rec2 = sp.tile([1, 128], dt.float32, tag="rec")
                nc.vector.reciprocal_approx_fast(out=rec2[:], in_=den2[:])
                rec2bf = sp.tile([1, 128], dt.bfloat16, tag="recbf")
                nc.vector.tensor_copy(rec2bf[:], rec2[:])
                rep2 = pp.tile([OUT_CH, 128], dt.float32, tag="trq")
                nc.tensor.matmul(rep2[:], lhsT=ones1[:], rhs=rec2bf[:], start=True, stop=True)
                rep2sb = sp.tile([OUT_CH, 128], dt.float32, tag="repsb")
                nc.scalar.copy(out=rep2sb[:], in_=rep2[:])
                o2 = sp.tile([OUT_CH, 128], dt.float32, tag="t1")
                nc.vector.tensor_tensor(out=o2[:], in0=ag2sb[0:OUT_CH, :],
                                        in1=rep2sb[:], op=AL.mult)
                otp = pp.tile([128, OUT_CH], dt.float32, tag="den")
                nc.tensor.transpose(out=otp[:], in_=o2[:],
                                    identity=idf[0:OUT_CH, 0:OUT_CH])
                osb = sp.tile([128, OUT_CH], dt.float32, tag="osb")
                nc.scalar.copy(out=osb[:], in_=otp[:])
                nc.sync.dma_start(out=OUT[b * 128:(b + 1) * 128, :], in_=osb[:])

    nc.compile()
    return nc


def _host_prep(x, edge_index, W1, att_src1, att_dst1, W2, att_src2, att_dst2,
               n_nodes, n_edges):
    NBLK = -(-n_nodes // BLK)
    NBLK = -(-NBLK // NCORES) * NCORES
    NP = NBLK * BLK
    BPC = NBLK // NCORES

    x = np.asarray(x, np.float32)
    W1 = np.asarray(W1, np.float32)
    W2 = np.asarray(W2, np.float32)
    att_src1 = np.asarray(att_src1, np.float32)
    att_dst1 = np.asarray(att_dst1, np.float32)
    att_src2 = np.asarray(att_src2, np.float32)
    att_dst2 = np.asarray(att_dst2, np.float32)
    H, C = att_src1.shape

    xp = np.zeros((NP, IN_CH), np.float32)
    xp[:n_nodes] = x
    XT = np.ascontiguousarray(xp.T).astype(bf16)          # [128, NP]

    Asrc1 = np.zeros((H * C, H), np.float32)
    Adst1 = np.zeros((H * C, H), np.float32)
    for h in range(H):
        Asrc1[h * C:(h + 1) * C, h] = att_src1[h]
        Adst1[h * C:(h + 1) * C, h] = att_dst1[h]
    W1A = np.concatenate([W1, W1 @ Asrc1, W1 @ Adst1], axis=1).astype(bf16)  # [128,136]
    W2A = np.concatenate([W2, W2 @ att_src2.T, W2 @ att_dst2.T], axis=1).astype(bf16)

    IOTA = np.broadcast_to(np.arange(128, dtype=np.float32), (128, 128)).astype(bf16)
    IDB = np.eye(128, dtype=np.float32).astype(bf16)
    IDF = np.eye(128, dtype=np.float32)
    HSEL = np.zeros((H, 128), np.float32)
    for h in range(H):
        HSEL[h, h * C:(h + 1) * C] = 1.0
    HSEL = HSEL.astype(bf16)
    ONES1 = np.ones((1, OUT_CH), np.float32).astype(bf16)

    src = np.asarray(edge_index[0], np.int64)
    dst = np.asarray(edge_index[1], np.int64)
    order = np.argsort(dst, kind="stable")
    ss = src[order]
    dd = dst[order]
    blk = dd // BLK
    bstart = np.searchsorted(blk, np.arange(NBLK))
    bcount = np.diff(np.append(bstart, n_edges))
    CPB = max(1, int(-(-bcount.max() // 128)))

    rank = np.arange(n_edges) - bstart[blk]
    core = blk // BPC
    b_in_core = blk % BPC
    chunk = rank // 128
    lane = rank % 128

    SRC1 = np.zeros((NCORES, BPC, 128, CPB), np.int32)
    SRC2 = np.zeros((NCORES, BPC, 128, CPB), np.int32)
    DREL = np.full((NCORES, BPC, 128, CPB), 200.0, np.float32)
    XTs = []
    for k in range(NCORES):
        sel = core == k
        bb, pp_, cc = b_in_core[sel], lane[sel], chunk[sel]
        s_k = ss[sel]
        rot = ((s_k // BLK - k * BPC) % NBLK) * BLK + (s_k % BLK)
        SRC1[k][bb, pp_, cc] = rot
        SRC2[k][bb, pp_, cc] = s_k
        DREL[k][bb, pp_, cc] = (dd[sel] % BLK).astype(np.float32)
        XTb = XT.reshape(128, NBLK, BLK)
        XTs.append(np.ascontiguousarray(
            np.roll(XTb, -k * BPC, axis=1).reshape(128, NP)))
    DREL = DREL.astype(bf16)

    consts = dict(w1a=W1A, w2a=W2A, iota=IOTA, idb=IDB, idf=IDF,
                  hsel=HSEL, ones1=ONES1)
    in_maps = []
    for k in range(NCORES):
        m = dict(consts)
        m["xt"] = XTs[k]
        m["src1"] = SRC1[k]
        m["src2"] = SRC2[k]
        m["drel"] = DREL[k]
        in_maps.append(m)
    return NP, NBLK, BPC, CPB, in_maps


_CACHE = {}


def _run(x, edge_index, W1, att_src1, att_dst1, W2, att_src2, att_dst2,
         n_nodes, n_edges, trace=False):
    from concourse import bass_utils
    NP, NBLK, BPC, CPB, in_maps = _host_prep(
        x, edge_index, W1, att_src1, att_dst1, W2, att_src2, att_dst2,
        n_nodes, n_edges)
    key = (NP, CPB)
    if key not in _CACHE:
        _CACHE[key] = _build(NP, NBLK, BPC, CPB)
    nc = _CACHE[key]
    res = bass_utils.run_bass_kernel_spmd(nc, in_maps, core_ids=list(range(NCORES)),
                                          trace=trace)
    out = np.concatenate([np.asarray(res.results[k]["out"]) for k in range(NCORES)],
                         axis=0)[:n_nodes]
    return np.ascontiguousarray(out.astype(np.float32)), res


def kernel(x, edge_index, W1, att_src1, att_dst1, W2, att_src2, att_dst2):
    out, _ = _run(x, edge_index, W1, att_src1, att_dst1, W2, att_src2, att_dst2,
                  N_NODES, N_EDGES)
    return out



# revision 2
# speedup vs baseline: 1.5827x; 1.5827x over previous
"""Bass/Trainium2 kernel v2 for 2-layer GAT (nn_GAT_50577534878113).

Strategy (8 NeuronCores, SPMD), v2 = batched-gather redesign:
  - dst-sorted edges sharded by dst block (49 blocks of 128 per core).
  - Dense phase replicated: PL1[NP,128] bf16 node table (h1 only) in DRAM.
  - Edge phase per dst block: ONE dma_gather per src-id span (A: id<32768,
    B: rest) pulls all the block's src rows (256B each) edge-major into SBUF.
    a_src1 per edge recomputed on DVE (mult by att row + 32-group reduce).
    Edges sorted by dst_rel within each span, so each dst owns a contiguous
    rank band: P^T is built with two DVE compare ops against an fp16 iota
    constant (no PE transposes, no PSUM->SBUF copies). a_dst routed to edges
    via per-chunk matmul with lhsT=P^T chunk. Scatter-add and denominators
    accumulate in PSUM via per-chunk matmuls with the edge-major one-hot P.
  - Layer 2: local dense fused per block; AllGather of [6272,64] bf16;
    PL2 viewed [25088,128] packs two nodes per 256B gather row (idx=src>>1,
    halves selected on-chip with a host mask folded into the alpha weights).
"""

import numpy as np
import ml_dtypes

bf16 = ml_dtypes.bfloat16
f16 = np.float16

N_NODES = 50000
N_EDGES = 800000
IN_CH = 128
HEADS = 4
HIDDEN = 32
OUT_CH = 40
NEG = 0.2
NCORES = 8
BLK = 128
SPLIT = 32768

F2 = OUT_CH + 2  # 42 used cols of the 64-col L2 row


def _build(NP, NBLK, BPC, CA, CB, CPB2, CL2):
    import concourse.bass as bass
    import concourse.bacc as bacc
    import concourse.mybir as mybir
    import concourse.tile as tile
    from concourse import library_config

    dt = mybir.dt
    AL = mybir.AluOpType
    AF = mybir.ActivationFunctionType

    CPB1 = CA + CB
    WA = CA * 128
    WB = CB * 128
    W2LO = CL2 * 128
    W2HI = (CPB2 - CL2) * 128

    nc = bacc.Bacc("TRN2", target_bir_lowering=False, debug=False,
                   num_devices=NCORES)

    XT = nc.dram_tensor("xt", [128, NP], dt.bfloat16, kind="ExternalInput").ap()
    W1A = nc.dram_tensor("w1a", [128, IN_CH + HEADS], dt.bfloat16, kind="ExternalInput").ap()
    W2A = nc.dram_tensor("w2a", [128, 64], dt.bfloat16, kind="ExternalInput").ap()
    IOTA = nc.dram_tensor("iota", [128, 128], dt.bfloat16, kind="ExternalInput").ap()
    IDF = nc.dram_tensor("idf", [128, 128], dt.float32, kind="ExternalInput").ap()
    HSEL = nc.dram_tensor("hsel", [HEADS, 128], dt.bfloat16, kind="ExternalInput").ap()
    ONES1 = nc.dram_tensor("ones1", [1, OUT_CH], dt.bfloat16, kind="ExternalInput").ap()
    ATTR = nc.dram_tensor("attr", [128, 128], dt.bfloat16, kind="ExternalInput").ap()
    ECON = nc.dram_tensor("econ", [128, 2048], dt.float16, kind="ExternalInput").ap()
    IDX1 = nc.dram_tensor("idx1", [BPC, 128, CPB1 * 8], dt.int16, kind="ExternalInput").ap()
    IDX2 = nc.dram_tensor("idx2", [BPC, 128, CPB2 * 8], dt.int16, kind="ExternalInput").ap()
    DREL1 = nc.dram_tensor("drel1", [BPC, 128, CPB1], dt.bfloat16, kind="ExternalInput").ap()
    DREL2 = nc.dram_tensor("drel2", [BPC, 128, CPB2], dt.bfloat16, kind="ExternalInput").ap()
    BND1 = nc.dram_tensor("bnd1", [BPC, 128, 4], dt.float16, kind="ExternalInput").ap()
    BND2 = nc.dram_tensor("bnd2", [BPC, 128, 4], dt.float16, kind="ExternalInput").ap()
    MSK2 = nc.dram_tensor("msk2", [BPC, 128, CPB2 * 2], dt.bfloat16, kind="ExternalInput").ap()
    OUT = nc.dram_tensor("out", [BPC * 128, OUT_CH], dt.float32, kind="ExternalOutput").ap()

    PL1 = nc.dram_tensor("pl1", [NP, 128], dt.bfloat16).ap()
    L2L = nc.dram_tensor("l2l", [BPC * 128, 64], dt.bfloat16).ap()
    PL2 = nc.dram_tensor("pl2", [NP // 2, 128], dt.bfloat16, addr_space="Shared").ap()

    with tile.TileContext(nc) as tc:
        with tc.tile_pool(name="const", bufs=1) as cp, \
             tc.tile_pool(name="sb", bufs=3) as sp, \
             tc.tile_pool(name="blk", bufs=2) as bp, \
             tc.tile_pool(name="ps", bufs=2, space="PSUM") as pp:

            iota = cp.tile([128, 128], dt.bfloat16)
            nc.sync.dma_start(out=iota[:], in_=IOTA[:])
            idf = cp.tile([128, 128], dt.float32)
            nc.sync.dma_start(out=idf[:], in_=IDF[:])
            hsel = cp.tile([HEADS, 128], dt.bfloat16)
            nc.sync.dma_start(out=hsel[:], in_=HSEL[:])
            ones1 = cp.tile([1, OUT_CH], dt.bfloat16)
            nc.sync.dma_start(out=ones1[:], in_=ONES1[:])
            attr = cp.tile([128, 128], dt.bfloat16)
            nc.sync.dma_start(out=attr[:], in_=ATTR[:])
            econ = cp.tile([128, 2048], dt.float16)
            nc.sync.dma_start(out=econ[:], in_=ECON[:])
            w1a = cp.tile([128, IN_CH + HEADS], dt.bfloat16)
            nc.sync.dma_start(out=w1a[:], in_=W1A[:])
            w2a = cp.tile([128, 64], dt.bfloat16)
            nc.sync.dma_start(out=w2a[:], in_=W2A[:])

            adst1 = cp.tile([128, BPC * HEADS], dt.bfloat16)
            adst2 = cp.tile([128, BPC], dt.bfloat16)
            elu1t = cp.tile([128, BPC * 128], dt.bfloat16)

            # ---- stage 1: dense h1 for ALL nodes -> PL1; a_dst1 for own ----
            XCH = 8
            for g0 in range(0, NBLK, XCH):
                xt_big = sp.tile([128, XCH * 128], dt.bfloat16, tag="xt")
                nc.sync.dma_start(out=xt_big[:],
                                  in_=XT[:, g0 * 128:(g0 + XCH) * 128])
                for j in range(XCH):
                    nb = g0 + j
                    dps = pp.tile([128, IN_CH + HEADS], dt.float32,
                                  tag=["agg", "den", "adg", "aux"][nb % 4])
                    nc.tensor.matmul(dps[:], lhsT=xt_big[:, j * 128:(j + 1) * 128],
                                     rhs=w1a[:], start=True, stop=True)
                    sb1 = sp.tile([128, 128], dt.bfloat16, tag=f"pl1sb{nb % 2}")
                    nc.scalar.copy(out=sb1[:], in_=dps[:, 0:128])
                    eng = nc.scalar if nb % 2 else nc.sync
                    eng.dma_start(out=PL1[nb * 128:(nb + 1) * 128, :], in_=sb1[:])
                    if nb < BPC:
                        nc.vector.tensor_copy(adst1[:, nb * HEADS:(nb + 1) * HEADS],
                                              dps[:, IN_CH:IN_CH + HEADS])

            # ---- stage 2: layer-1 edge pass ----
            for b in range(BPC):
                idx_t = sp.tile([128, CPB1 * 8], dt.int16, tag="idx")
                nc.sync.dma_start(out=idx_t[:], in_=IDX1[b])
                dr_t = sp.tile([128, CPB1], dt.bfloat16, tag="dr")
                nc.sync.dma_start(out=dr_t[:], in_=DREL1[b])
                bnd = sp.tile([128, 4], dt.float16, tag="bnd")
                nc.sync.dma_start(out=bnd[:], in_=BND1[b])

                g = bp.tile([128, CPB1 * 128], dt.bfloat16, tag="g")

                nc.gpsimd.dma_gather(
                    g[:, 0:WA].rearrange("p (c f) -> p c f", f=128),
                    PL1[:], idx_t[:, 0:CA * 8], WA, WA, 128,
                    queue_num=b % 4)
                nc.gpsimd.dma_gather(
                    g[:, WA:].rearrange("p (c f) -> p c f", f=128),
                    PL1[SPLIT:, :], idx_t[:, CA * 8:], WB, WB, 128,
                    queue_num=(b + 1) % 4)

                # one-hot P[e, (c d)] = (drel[e,c] == d)
                P = bp.tile([128, CPB1 * 128], dt.bfloat16, tag="P")
                nc.vector.tensor_tensor(
                    out=P[:].rearrange("p (c j) -> p c j", j=128),
                    in0=dr_t[:, :, None].broadcast_to([128, CPB1, 128]),
                    in1=iota[:, None, :].broadcast_to([128, CPB1, 128]),
                    op=AL.is_equal)

                # banded P^T[d, e] via rank-interval compares (per span)
                PT = bp.tile([128, CPB1 * 128], dt.bfloat16, tag="PT")
                aA = bp.tile([128, WA], dt.bfloat16, tag="aA")
                nc.vector.tensor_tensor(
                    out=aA[:], in0=econ[:, 0:WA],
                    in1=bnd[:, 0:1].broadcast_to([128, WA]), op=AL.is_ge)
                nc.vector.scalar_tensor_tensor(
                    out=PT[:, 0:WA], in0=econ[:, 0:WA], scalar=bnd[:, 1:2],
                    in1=aA[:], op0=AL.is_lt, op1=AL.mult)
                aB = bp.tile([128, WB], dt.bfloat16, tag="aB")
                nc.vector.tensor_tensor(
                    out=aB[:], in0=econ[:, 0:WB],
                    in1=bnd[:, 2:3].broadcast_to([128, WB]), op=AL.is_ge)
                nc.vector.scalar_tensor_tensor(
                    out=PT[:, WA:], in0=econ[:, 0:WB], scalar=bnd[:, 3:4],
                    in1=aB[:], op0=AL.is_lt, op1=AL.mult)

                # a_src per edge: (g * att_row) summed per 32-group
                tmp = bp.tile([128, CPB1 * 128], dt.bfloat16, tag="tmp")
                nc.vector.tensor_tensor(
                    out=tmp[:].rearrange("p (c h w) -> p c h w", h=HEADS, w=HIDDEN),
                    in0=g[:].rearrange("p (c h w) -> p c h w", h=HEADS, w=HIDDEN),
                    in1=attr[:].rearrange("p (h w) -> p h w", w=HIDDEN)[:, None]
                        .broadcast_to([128, CPB1, HEADS, HIDDEN]),
                    op=AL.mult)
                asrcE = sp.tile([128, CPB1 * HEADS], dt.float32, tag="asrc")
                nc.vector.tensor_reduce(
                    out=asrcE[:].rearrange("p (c h) -> p c h", h=HEADS),
                    in_=tmp[:].rearrange("p (c h w) -> p c h w", h=HEADS, w=HIDDEN),
                    axis=mybir.AxisListType.X, op=AL.add)

                # a_dst routed to edges: per-chunk matmul with lhsT = P^T chunk
                ADG = pp.tile([128, CPB1 * HEADS], dt.float32, tag="adg")
                for c in range(CPB1):
                    nc.tensor.matmul(ADG[:, c * HEADS:(c + 1) * HEADS],
                                     lhsT=PT[:, c * 128:(c + 1) * 128],
                                     rhs=adst1[:, b * HEADS:(b + 1) * HEADS],
                                     start=True, stop=True)

                elog = sp.tile([128, CPB1 * HEADS], dt.float32, tag="elog")
                nc.vector.tensor_tensor(out=elog[:], in0=asrcE[:], in1=ADG[:], op=AL.add)
                lr = sp.tile([128, CPB1 * HEADS], dt.float32, tag="lr")
                nc.vector.scalar_tensor_tensor(out=lr[:], in0=elog[:], scalar=NEG,
                                               in1=elog[:], op0=AL.mult, op1=AL.max)
                s_all = sp.tile([128, CPB1 * HEADS], dt.bfloat16, tag="sall")
                nc.scalar.activation(out=s_all[:], in_=lr[:], func=AF.Exp)

                gs = bp.tile([128, CPB1 * 128], dt.bfloat16, tag="gs")
                nc.vector.tensor_tensor(
                    out=gs[:].rearrange("p (c h w) -> p c h w", h=HEADS, w=HIDDEN),
                    in0=g[:].rearrange("p (c h w) -> p c h w", h=HEADS, w=HIDDEN),
                    in1=s_all[:].rearrange("p (c h) -> p c h", h=HEADS)[:, :, :, None]
                        .broadcast_to([128, CPB1, HEADS, HIDDEN]),
                    op=AL.mult)

                AGG = pp.tile([128, 128], dt.float32, tag="agg")
                DEN = pp.tile([HEADS, 128], dt.float32, tag="den")
                for c in range(CPB1):
                    nc.tensor.matmul(AGG[:], lhsT=gs[:, c * 128:(c + 1) * 128],
                                     rhs=P[:, c * 128:(c + 1) * 128],
                                     start=(c == 0), stop=(c == CPB1 - 1))
                    nc.tensor.matmul(DEN[:], lhsT=s_all[:, c * HEADS:(c + 1) * HEADS],
                                     rhs=P[:, c * 128:(c + 1) * 128],
                                     start=(c == 0), stop=(c == CPB1 - 1))

                # normalize + ELU, store transposed block to elu1t
                den_sb = sp.tile([HEADS, 128], dt.float32, tag="densb")
                nc.vector.tensor_scalar(out=den_sb[:], in0=DEN[:], scalar1=1e-16,
                                        scalar2=None, op0=AL.add)
                rec = sp.tile([HEADS, 128], dt.float32, tag="rec")
                nc.vector.reciprocal_approx_fast(out=rec[:], in_=den_sb[:])
                rec_bf = sp.tile([HEADS, 128], dt.bfloat16, tag="recbf")
                nc.vector.tensor_copy(rec_bf[:], rec[:])
                rep = pp.tile([128, 128], dt.float32, tag="aux")
                nc.tensor.matmul(rep[:], lhsT=hsel[:], rhs=rec_bf[:], start=True, stop=True)
                rep_sb = sp.tile([128, 128], dt.float32, tag="repsb")
                nc.scalar.copy(out=rep_sb[:], in_=rep[:])
                t1 = sp.tile([128, 128], dt.float32, tag="t1")
                nc.vector.tensor_tensor(out=t1[:], in0=AGG[:], in1=rep_sb[:], op=AL.mult)
                m1 = sp.tile([128, 128], dt.float32, tag="m1")
                nc.vector.tensor_scalar(out=m1[:], in0=t1[:], scalar1=0.0,
                                        scalar2=None, op0=AL.min)
                u1 = sp.tile([128, 128], dt.float32, tag="u1")
                nc.scalar.activation(out=u1[:], in_=m1[:], func=AF.Exp)
                pm1 = sp.tile([128, 128], dt.float32, tag="pm1")
                nc.vector.tensor_scalar(out=pm1[:], in0=t1[:], scalar1=0.0, scalar2=-1.0,
                                        op0=AL.max, op1=AL.add)
                nc.vector.tensor_tensor(out=elu1t[:, b * 128:(b + 1) * 128],
                                        in0=pm1[:], in1=u1[:], op=AL.add)

                # fused layer-2 local dense for this block
                d2 = pp.tile([128, 64], dt.float32, tag="adg")
                nc.tensor.matmul(d2[:], lhsT=elu1t[:, b * 128:(b + 1) * 128],
                                 rhs=w2a[:], start=True, stop=True)
                sb2 = sp.tile([128, 64], dt.bfloat16, tag="sb2")
                nc.scalar.copy(out=sb2[:], in_=d2[:])
                nc.scalar.dma_start(out=L2L[b * 128:(b + 1) * 128, :], in_=sb2[:])
                nc.vector.tensor_copy(adst2[:, b:b + 1], d2[:, F2 - 1:F2])

            nc.gpsimd.collective_compute(
                "AllGather", mybir.AluOpType.bypass,
                replica_groups=[list(range(NCORES))],
                ins=[L2L[:, :]], outs=[PL2[:, :]])

            # ---- stage 3: layer-2 edge pass ----
            for b in range(BPC):
                idx2 = sp.tile([128, CPB2 * 8], dt.int16, tag="idx")
                nc.sync.dma_start(out=idx2[:], in_=IDX2[b])
                dr2 = sp.tile([128, CPB2], dt.bfloat16, tag="dr")
                nc.sync.dma_start(out=dr2[:], in_=DREL2[b])
                bnd2 = sp.tile([128, 4], dt.float16, tag="bnd")
                nc.sync.dma_start(out=bnd2[:], in_=BND2[b])
                msk = sp.tile([128, CPB2 * 2], dt.bfloat16, tag="msk")
                nc.sync.dma_start(out=msk[:], in_=MSK2[b])

                g2 = bp.tile([128, CPB2 * 128], dt.bfloat16, tag="g")
                nc.gpsimd.dma_gather(
                    g2[:].rearrange("p (c f) -> p c f", f=128),
                    PL2[:], idx2[:], CPB2 * 128, CPB2 * 128, 128,
                    queue_num=b % 4)

                P2 = bp.tile([128, CPB2 * 128], dt.bfloat16, tag="P")
                nc.vector.tensor_tensor(
                    out=P2[:].rearrange("p (c j) -> p c j", j=128),
                    in0=dr2[:, :, None].broadcast_to([128, CPB2, 128]),
                    in1=iota[:, None, :].broadcast_to([128, CPB2, 128]),
                    op=AL.is_equal)

                PT2 = bp.tile([128, CPB2 * 128], dt.bfloat16, tag="PT")
                aL = bp.tile([128, W2LO], dt.bfloat16, tag="aA")
                nc.vector.tensor_tensor(
                    out=aL[:], in0=econ[:, 0:W2LO],
                    in1=bnd2[:, 0:1].broadcast_to([128, W2LO]), op=AL.is_ge)
                nc.vector.scalar_tensor_tensor(
                    out=PT2[:, 0:W2LO], in0=econ[:, 0:W2LO], scalar=bnd2[:, 1:2],
                    in1=aL[:], op0=AL.is_lt, op1=AL.mult)
                aH = bp.tile([128, W2HI], dt.bfloat16, tag="aB")
                nc.vector.tensor_tensor(
                    out=aH[:], in0=econ[:, 0:W2HI],
                    in1=bnd2[:, 2:3].broadcast_to([128, W2HI]), op=AL.is_ge)
                nc.vector.scalar_tensor_tensor(
                    out=PT2[:, W2LO:], in0=econ[:, 0:W2HI], scalar=bnd2[:, 3:4],
                    in1=aH[:], op0=AL.is_lt, op1=AL.mult)

                # a_src2 per edge: select col 40 of the gathered half
                g2v = g2[:].rearrange("p (c f) -> p c f", f=128)
                mv = msk[:].rearrange("p (c t) -> p c t", t=2)
                df2 = sp.tile([128, CPB2], dt.bfloat16, tag="df2")
                nc.vector.tensor_tensor(out=df2[:, :, None], in0=g2v[:, :, 104:105],
                                        in1=g2v[:, :, 40:41], op=AL.subtract)
                tsel = sp.tile([128, CPB2], dt.bfloat16, tag="tsel")
                nc.vector.tensor_tensor(out=tsel[:, :, None], in0=df2[:, :, None],
                                        in1=mv[:, :, 1:2], op=AL.mult)
                asrc2 = sp.tile([128, CPB2], dt.float32, tag="asrc2")
                nc.vector.tensor_tensor(out=asrc2[:, :, None], in0=tsel[:, :, None],
                                        in1=g2v[:, :, 40:41], op=AL.add)

                ADG2 = pp.tile([128, CPB2], dt.float32, tag="adg")
                for c in range(CPB2):
                    nc.tensor.matmul(ADG2[:, c:c + 1],
                                     lhsT=PT2[:, c * 128:(c + 1) * 128],
                                     rhs=adst2[:, b:b + 1], start=True, stop=True)

                elog2 = sp.tile([128, CPB2], dt.float32, tag="elog")
                nc.vector.tensor_tensor(out=elog2[:], in0=asrc2[:], in1=ADG2[:], op=AL.add)
                lr2 = sp.tile([128, CPB2], dt.float32, tag="lr")
                nc.vector.scalar_tensor_tensor(out=lr2[:], in0=elog2[:], scalar=NEG,
                                               in1=elog2[:], op0=AL.mult, op1=AL.max)
                s2b = sp.tile([128, CPB2], dt.bfloat16, tag="s2b")
                nc.scalar.activation(out=s2b[:], in_=lr2[:], func=AF.Exp)

                s2m = sp.tile([128, CPB2 * 2], dt.bfloat16, tag="s2m")
                nc.vector.tensor_tensor(
                    out=s2m[:].rearrange("p (c t) -> p c t", t=2),
                    in0=s2b[:, :, None].broadcast_to([128, CPB2, 2]),
                    in1=mv, op=AL.mult)

                gs2 = bp.tile([128, CPB2 * 128], dt.bfloat16, tag="gs")
                nc.vector.tensor_tensor(
                    out=gs2[:].rearrange("p (c t f) -> p c t f", t=2, f=64),
                    in0=g2[:].rearrange("p (c t f) -> p c t f", t=2, f=64),
                    in1=s2m[:].rearrange("p (c t) -> p c t", t=2)[:, :, :, None]
                        .broadcast_to([128, CPB2, 2, 64]),
                    op=AL.mult)

                AGG2 = pp.tile([128, 128], dt.float32, tag="agg")
                DEN2 = pp.tile([1, 128], dt.float32, tag="den")
                for c in range(CPB2):
                    nc.tensor.matmul(AGG2[:], lhsT=gs2[:, c * 128:(c + 1) * 128],
                                     rhs=P2[:, c * 128:(c + 1) * 128],
                                     start=(c == 0), stop=(c == CPB2 - 1))
                    nc.tensor.matmul(DEN2[:], lhsT=s2b[:, c:c + 1],
                                     rhs=P2[:, c * 128:(c + 1) * 128],
                                     start=(c == 0), stop=(c == CPB2 - 1))

                o2a = sp.tile([OUT_CH, 128], dt.float32, tag="o2a")
                nc.scalar.copy(out=o2a[:], in_=AGG2[0:OUT_CH, :])
                o2pre = sp.tile([OUT_CH, 128], dt.float32, tag="o2pre")
                nc.vector.tensor_tensor(out=o2pre[:], in0=o2a[:],
                                        in1=AGG2[64:64 + OUT_CH, :], op=AL.add)

                den2 = sp.tile([1, 128], dt.float32, tag="densb")
                nc.vector.tensor_scalar(out=den2[:], in0=DEN2[:],
                                        scalar1=1e-16, scalar2=None, op0=AL.add)
                rec2 = sp.tile([1, 128], dt.float32, tag="rec")
                nc.vector.reciprocal_approx_fast(out=rec2[:], in_=den2[:])
                rec2bf = sp.tile([1, 128], dt.bfloat16, tag="recbf")
                nc.vector.tensor_copy(rec2bf[:], rec2[:])
                rep2 = pp.tile([OUT_CH, 128], dt.float32, tag="aux")
                nc.tensor.matmul(rep2[:], lhsT=ones1[:], rhs=rec2bf[:], start=True, stop=True)
                rep2sb = sp.tile([OUT_CH, 128], dt.float32, tag="repsb")
                nc.scalar.copy(out=rep2sb[:], in_=rep2[:])
                o2 = sp.tile([OUT_CH, 128], dt.float32, tag="t1")
                nc.vector.tensor_tensor(out=o2[:], in0=o2pre[:], in1=rep2sb[:], op=AL.mult)
                otp = pp.tile([128, OUT_CH], dt.float32, tag="den")
                nc.tensor.transpose(out=otp[:], in_=o2[:],
                                    identity=idf[0:OUT_CH, 0:OUT_CH])
                osb = sp.tile([128, OUT_CH], dt.float32, tag="osb")
                nc.scalar.copy(out=osb[:], in_=otp[:])
                nc.sync.dma_start(out=OUT[b * 128:(b + 1) * 128, :], in_=osb[:])

    nc.compile()
    return nc


def _wrap_idx(flat, n):
    """flat int list (len n, multiple of 128) -> [128, n//16] wrapped+replicated."""
    a = np.asarray(flat, np.int16).reshape(n // 16, 16).T  # [16, n//16]
    return np.tile(a, (8, 1))


def _host_prep(x, edge_index, W1, att_src1, att_dst1, W2, att_src2, att_dst2,
               n_nodes, n_edges):
    NBLK = -(-n_nodes // BLK)
    NBLK = -(-NBLK // NCORES) * NCORES
    NP = NBLK * BLK
    BPC = NBLK // NCORES

    x = np.asarray(x, np.float32)
    W1 = np.asarray(W1, np.float32)
    W2 = np.asarray(W2, np.float32)
    att_src1 = np.asarray(att_src1, np.float32)
    att_dst1 = np.asarray(att_dst1, np.float32)
    att_src2 = np.asarray(att_src2, np.float32)
    att_dst2 = np.asarray(att_dst2, np.float32)
    H, C = att_src1.shape

    xp = np.zeros((NP, IN_CH), np.float32)
    xp[:n_nodes] = x
    XT = np.ascontiguousarray(xp.T).astype(bf16)          # [128, NP]

    Adst1 = np.zeros((H * C, H), np.float32)
    for h in range(H):
        Adst1[h * C:(h + 1) * C, h] = att_dst1[h]
    W1A = np.concatenate([W1, W1 @ Adst1], axis=1).astype(bf16)  # [128,132]
    W2A = np.zeros((H * C, 64), np.float32)
    W2A[:, 0:OUT_CH] = W2
    W2A[:, OUT_CH:OUT_CH + 1] = W2 @ att_src2.T
    W2A[:, OUT_CH + 1:OUT_CH + 2] = W2 @ att_dst2.T
    W2A = W2A.astype(bf16)

    IOTA = np.broadcast_to(np.arange(128, dtype=np.float32), (128, 128)).astype(bf16)
    IDF = np.eye(128, dtype=np.float32)
    HSEL = np.zeros((H, 128), np.float32)
    for h in range(H):
        HSEL[h, h * C:(h + 1) * C] = 1.0
    HSEL = HSEL.astype(bf16)
    ONES1 = np.ones((1, OUT_CH), np.float32).astype(bf16)
    ATTR = np.broadcast_to(att_src1.reshape(-1), (128, 128)).astype(bf16)
    ECON = np.broadcast_to(np.arange(2048, dtype=np.float32), (128, 2048)).astype(f16)

    src = np.asarray(edge_index[0], np.int64)
    dst = np.asarray(edge_index[1], np.int64)
    order = np.argsort(dst, kind="stable")
    ss = src[order]
    dd = dst[order]
    blk = dd // BLK
    drel = (dd % BLK).astype(np.int64)
    core = blk // BPC
    rot = ((ss // BLK - core * BPC) % NBLK) * BLK + (ss % BLK)
    isB = rot >= SPLIT
    idx2g = ss >> 1
    m2 = (ss & 1).astype(np.float32)

    bstart = np.searchsorted(blk, np.arange(NBLK))
    bend = np.append(bstart[1:], n_edges)

    # global span maxima
    nA = np.bincount(blk * 2 + isB, minlength=2 * NBLK)[0::2]
    nB = np.bincount(blk * 2 + isB, minlength=2 * NBLK)[1::2]
    CA = max(1, int(-(-nA.max() // 128)))
    CB = max(1, int(-(-nB.max() // 128)))
    CPB2 = max(1, int(-(-(bend - bstart).max() // 128)))
    CL2 = (CPB2 + 1) // 2
    assert CA * 128 <= 2048 and CB * 128 <= 2048
    assert CL2 * 128 <= 2048 and (CPB2 - CL2) * 128 <= 2048
    CPB1 = CA + CB

    IDX1 = np.zeros((NCORES, BPC, 128, CPB1 * 8), np.int16)
    IDX2 = np.zeros((NCORES, BPC, 128, CPB2 * 8), np.int16)
    DREL1 = np.full((NCORES, BPC, 128, CPB1), 200.0, np.float32)
    DREL2 = np.full((NCORES, BPC, 128, CPB2), 200.0, np.float32)
    BND1 = np.zeros((NCORES, BPC, 128, 4), np.float32)
    BND2 = np.zeros((NCORES, BPC, 128, 4), np.float32)
    MSK2 = np.zeros((NCORES, BPC, 128, CPB2 * 2), np.float32)
    dgrid = np.arange(128)

    for gblk in range(NBLK):
        k, b = gblk // BPC, gblk % BPC
        e0, e1 = bstart[gblk], bend[gblk]
        dl = drel[e0:e1]
        rl = rot[e0:e1]
        Bm = isB[e0:e1]
        # L1 span A
        oA = np.argsort(dl[~Bm], kind="stable")
        iA = rl[~Bm][oA]
        dA = dl[~Bm][oA]
        na = len(iA)
        padA = np.zeros(CA * 128, np.int64)
        padA[:na] = iA
        dfA = np.full((CA, 128), 200.0, np.float32)
        dfA.reshape(-1)[:na] = dA
        # L1 span B
        oB = np.argsort(dl[Bm], kind="stable")
        iB = rl[Bm][oB] - SPLIT
        dB = dl[Bm][oB]
        nb_ = len(iB)
        padB = np.zeros(CB * 128, np.int64)
        padB[:nb_] = iB
        dfB = np.full((CB, 128), 200.0, np.float32)
        dfB.reshape(-1)[:nb_] = dB
        IDX1[k, b] = np.hstack([_wrap_idx(padA, CA * 128), _wrap_idx(padB, CB * 128)])
        DREL1[k, b] = np.hstack([dfA.T, dfB.T])
        BND1[k, b, :, 0] = np.searchsorted(dA, dgrid, "left")
        BND1[k, b, :, 1] = np.searchsorted(dA, dgrid, "right")
        BND1[k, b, :, 2] = np.searchsorted(dB, dgrid, "left")
        BND1[k, b, :, 3] = np.searchsorted(dB, dgrid, "right")
        # L2 (single sorted list, lo/hi spans)
        o2 = np.argsort(dl, kind="stable")
        i2 = idx2g[e0:e1][o2]
        d2v = dl[o2]
        mm = m2[e0:e1][o2]
        n2 = len(i2)
        pad2 = np.zeros(CPB2 * 128, np.int64)
        pad2[:n2] = i2
        df2 = np.full((CPB2, 128), 200.0, np.float32)
        df2.reshape(-1)[:n2] = d2v
        mf2 = np.zeros((CPB2, 128), np.float32)
        mf2.reshape(-1)[:n2] = mm
        IDX2[k, b] = _wrap_idx(pad2, CPB2 * 128)
        DREL2[k, b] = df2.T
        mpair = np.stack([1.0 - mf2.T, mf2.T], axis=2)  # [128, CPB2, 2]
        MSK2[k, b] = mpair.reshape(128, CPB2 * 2)
        s_full = np.searchsorted(d2v, dgrid, "left")
        e_full = np.searchsorted(d2v, dgrid, "right")
        HL = CL2 * 128
        BND2[k, b, :, 0] = np.clip(s_full, 0, HL)
        BND2[k, b, :, 1] = np.clip(e_full, 0, HL)
        BND2[k, b, :, 2] = np.clip(s_full - HL, 0, CPB2 * 128 - HL)
        BND2[k, b, :, 3] = np.clip(e_full - HL, 0, CPB2 * 128 - HL)

    consts = dict(w1a=W1A, w2a=W2A, iota=IOTA, idf=IDF, hsel=HSEL,
                  ones1=ONES1, attr=ATTR, econ=ECON)
    XTb = XT.reshape(128, NBLK, BLK)
    in_maps = []
    for k in range(NCORES):
        m = dict(consts)
        m["xt"] = np.ascontiguousarray(np.roll(XTb, -k * BPC, axis=1).reshape(128, NP))
        m["idx1"] = IDX1[k]
        m["idx2"] = IDX2[k]
        m["drel1"] = DREL1[k].astype(bf16)
        m["drel2"] = DREL2[k].astype(bf16)
        m["bnd1"] = BND1[k].astype(f16)
        m["bnd2"] = BND2[k].astype(f16)
        m["msk2"] = MSK2[k].astype(bf16)
        in_maps.append(m)
    return NP, NBLK, BPC, CA, CB, CPB2, CL2, in_maps


_CACHE = {}


def _run(x, edge_index, W1, att_src1, att_dst1, W2, att_src2, att_dst2,
         n_nodes, n_edges, trace=False):
    from concourse import bass_utils
    NP, NBLK, BPC, CA, CB, CPB2, CL2, in_maps = _host_prep(
        x, edge_index, W1, att_src1, att_dst1, W2, att_src2, att_dst2,
        n_nodes, n_edges)
    key = (NP, CA, CB, CPB2)
    if key not in _CACHE:
        _CACHE[key] = _build(NP, NBLK, BPC, CA, CB, CPB2, CL2)
    nc = _CACHE[key]
    res = bass_utils.run_bass_kernel_spmd(nc, in_maps, core_ids=list(range(NCORES)),
                                          trace=trace)
    out = np.concatenate([np.asarray(res.results[k]["out"]) for k in range(NCORES)],
                         axis=0)[:n_nodes]
    return np.ascontiguousarray(out.astype(np.float32)), res


def kernel(x, edge_index, W1, att_src1, att_dst1, W2, att_src2, att_dst2):
    out, _ = _run(x, edge_index, W1, att_src1, att_dst1, W2, att_src2, att_dst2,
                  N_NODES, N_EDGES)
    return out
